# revision 5
# baseline (speedup 1.0000x reference)
"""Trainium2 Bass kernel for a single masked attention head.

Problem: B=8, S=2048, DIM_IN=768, DIM_K=DIM_V=64.
  q = query @ W_q.T + b_q ; k = key @ W_k.T + b_k ; v = value @ W_v.T + b_v
  scores = (q @ k.T) / 8 ; scores[mask] = -inf ; out = softmax(scores) @ v

Sharding: data-parallel over batch — one batch element per NeuronCore (8 cores).

Per-core dataflow (everything stays feature-major to keep the softmax
reduction off the partition axis):
  1. PE-transpose input chunks X[s,768] -> X.T[i,s], project with W.T chunks:
     qT,kT,vT [64, S] (feature-major).
  2. Scores transposed: S.T[b,a] = kT[:,b].T @ qT[:,a]  (contract over 64).
     Key-mask is then a per-partition value -> folded into the ACT exp bias.
  3. P.T = exp(S.T/8 + maskbias)  (no row-max: scores are O(+-8), fp32-safe;
     masked lanes get bias -1e4 -> exp underflows to exactly 0).
  4. O_aug.T[65,a] = sum_b V_aug[b,:65].T @ P.T[b,a] with V_aug = [V | 1];
     row 64 accumulates the softmax denominator.
  5. PE-transpose O_aug.T back to [a,65], multiply rows by 1/col64, DMA out.
"""

import numpy as np

S = 2048
DIN = 768
DK = 64
NB = S // 128    # key chunks (partition-dim blocks)
NI = DIN // 128  # feature chunks
NA = S // 512    # query tiles
MASK_NEG = -10000.0

_CACHE = {}


def build_nc(s=S, mm_dtype="float32r"):
    import concourse.bacc as bacc
    import concourse.mybir as mybir
    import concourse.tile as tile
    from concourse.masks import make_identity

    f32 = mybir.dt.float32
    mmdt = getattr(mybir.dt, mm_dtype)
    nb, na = s // 128, s // 512

    nc = bacc.Bacc("TRN2", target_bir_lowering=False, debug=False)

    xq_d = nc.dram_tensor("xq", [s, DIN], f32, kind="ExternalInput")
    xk_d = nc.dram_tensor("xk", [s, DIN], f32, kind="ExternalInput")
    xv_d = nc.dram_tensor("xv", [s, DIN], f32, kind="ExternalInput")
    mask_d = nc.dram_tensor("mask", [1, s], mybir.dt.uint8, kind="ExternalInput")
    wq_d = nc.dram_tensor("wq", [DK, DIN], f32, kind="ExternalInput")
    wk_d = nc.dram_tensor("wk", [DK, DIN], f32, kind="ExternalInput")
    wv_d = nc.dram_tensor("wv", [DK, DIN], f32, kind="ExternalInput")
    bq_d = nc.dram_tensor("bq", [1, DK], f32, kind="ExternalInput")
    bk_d = nc.dram_tensor("bk", [1, DK], f32, kind="ExternalInput")
    bv_d = nc.dram_tensor("bv", [1, DK], f32, kind="ExternalInput")
    out_d = nc.dram_tensor("out", [s, DK], f32, kind="ExternalOutput")


    with tile.TileContext(nc) as tc:
        with (
            tc.tile_pool(name="const", bufs=1) as cp,
            tc.tile_pool(name="xstage", bufs=3) as xp,
            tc.tile_pool(name="xt", bufs=2) as xtp,
            tc.tile_pool(name="pt", bufs=2) as ptp,
            tc.tile_pool(name="osb", bufs=2) as osp,
            tc.tile_pool(name="ps_small", bufs=3, space="PSUM") as ps_small,
            tc.tile_pool(name="ps_proj", bufs=2, space="PSUM") as ps_proj,
            tc.tile_pool(name="ps_st", bufs=2, space="PSUM") as ps_st,
            tc.tile_pool(name="ps_ot", bufs=1, space="PSUM") as ps_ot,
        ):
            # ---- setup: identity, weights transposed, biases, mask bias ----
            ident = cp.tile([128, 128], f32)
            make_identity(nc, ident[:])

            consts = cp.tile([1, 2], f32)
            nc.vector.memset(consts[:, 0:1], MASK_NEG)
            nc.vector.memset(consts[:, 1:2], 1.0)

            wts = {}
            biases = {}
            for name, w_d, b_d in (
                ("q", wq_d, bq_d), ("k", wk_d, bk_d), ("v", wv_d, bv_d),
            ):
                w_sb = xp.tile([DK, DIN], f32, tag="wload")
                nc.sync.dma_start(w_sb[:], w_d.ap())
                wt = cp.tile([128, NI, DK], mmdt, tag=f"wt_{name}")
                for i in range(NI):
                    tp = ps_small.tile([128, DK], f32, tag="tp")
                    nc.tensor.transpose(
                        tp[:], w_sb[:, i * 128:(i + 1) * 128], ident[:DK, :DK]
                    )
                    nc.any.tensor_copy(wt[:, i, :], tp[:])
                wts[name] = wt

                b_sb = cp.tile([1, DK], f32, tag=f"bld_{name}")
                nc.sync.dma_start(b_sb[:], b_d.ap())
                bp = ps_small.tile([DK, 1], f32, tag="tp")
                nc.tensor.matmul(bp[:], b_sb[:], consts[:, 1:2])
                bt = cp.tile([DK, 1], f32, tag=f"b_{name}")
                nc.any.tensor_copy(bt[:], bp[:])
                biases[name] = bt

            # mask [1, s] u8 -> f32 -> per-chunk [128,1] bias (0 / MASK_NEG)
            mask_u8 = cp.tile([1, s], mybir.dt.uint8)
            nc.sync.dma_start(mask_u8[:], mask_d.ap())
            mask_f = cp.tile([1, s], f32)
            nc.vector.tensor_copy(mask_f[:], mask_u8[:])
            mb_ps = ps_small.tile([128, nb], f32, tag="tp")
            for j in range(nb):
                nc.tensor.matmul(
                    mb_ps[:, j:j + 1],
                    mask_f[:, j * 128:(j + 1) * 128],
                    consts[:, 0:1],
                )
            maskb = cp.tile([128, nb], f32)
            nc.any.tensor_copy(maskb[:], mb_ps[:])

            # ---- phase 1: transpose + project -> qT, kT, vT [64, s] ----
            projT = {}
            for name, x_d in (("q", xq_d), ("k", xk_d), ("v", xv_d)):
                tdt = f32 if name == "v" else mmdt
                tT = cp.tile([DK, s], tdt, tag=f"pt_{name}")
                projT[name] = tT
                for a in range(na):
                    xt = xtp.tile([128, NI, 512], mmdt, tag="xt")
                    for ss in range(4):
                        x_sb = xp.tile([128, DIN], f32, tag="xload")
                        r0 = a * 512 + ss * 128
                        nc.sync.dma_start(x_sb[:], x_d.ap()[r0:r0 + 128, :])
                        for i in range(NI):
                            tp = ps_small.tile([128, 128], f32, tag="tp")
                            nc.tensor.transpose(
                                tp[:], x_sb[:, i * 128:(i + 1) * 128], ident[:]
                            )
                            nc.any.tensor_copy(
                                xt[:, i, ss * 128:(ss + 1) * 128], tp[:]
                            )
                    pj = ps_proj.tile([DK, 512], f32, tag="proj")
                    for i in range(NI):
                        nc.tensor.matmul(
                            pj[:], wts[name][:, i, :], xt[:, i, :],
                            start=(i == 0), stop=(i == NI - 1),
                        )
                    nc.vector.tensor_scalar_add(
                        tT[:, a * 512:(a + 1) * 512], pj[:], biases[name][:]
                    )

            # V natural layout with ones column: vaug[:, j, :] = [V_chunk | 1]
            vaug = cp.tile([128, nb, DK + 2], mmdt)
            ones_f = cp.tile([128, 2], f32)
            nc.vector.memset(ones_f[:, 0:1], 1.0)
            nc.vector.memset(ones_f[:, 1:2], 0.0)
            for j in range(nb):
                nc.vector.tensor_copy(vaug[:, j, DK:DK + 2], ones_f[:])
            for j in range(nb):
                tp = ps_small.tile([128, DK], f32, tag="tp")
                nc.tensor.transpose(
                    tp[:], projT["v"][:, j * 128:(j + 1) * 128], ident[:DK, :DK]
                )
                nc.any.tensor_copy(vaug[:, j, 0:DK], tp[:])

            # ---- phase 2: scores.T -> exp -> PV -> transpose -> normalize ----
            for a in range(na):
                pt = ptp.tile([128, nb, 512], mmdt, tag="pt")
                qa = projT["q"][:, a * 512:(a + 1) * 512]
                for j in range(nb):
                    st = ps_st.tile([128, 512], f32, tag="st")
                    nc.tensor.matmul(
                        st[:], projT["k"][:, j * 128:(j + 1) * 128], qa
                    )
                    nc.scalar.activation(
                        pt[:, j, :], st[:],
                        mybir.ActivationFunctionType.Exp,
                        bias=maskb[:, j:j + 1], scale=0.125,
                    )
                ot = ps_ot.tile([DK + 2, 512], f32, tag="ot")
                for j in range(nb):
                    nc.tensor.matmul(
                        ot[:], vaug[:, j, :], pt[:, j, :],
                        start=(j == 0), stop=(j == nb - 1),
                    )
                ot_sb = osp.tile([DK + 2, 512], f32, tag="ot_sb")
                nc.any.tensor_copy(ot_sb[:], ot[:])
                for ss in range(4):
                    op = ps_small.tile([128, DK + 2], f32, tag="tp")
                    nc.tensor.transpose(
                        op[:], ot_sb[:, ss * 128:(ss + 1) * 128],
                        ident[:DK + 2, :DK + 2],
                    )
                    rcp = osp.tile([128, 1], f32, tag="rcp")
                    nc.vector.reciprocal(rcp[:], op[:, DK:DK + 1])
                    o_sb = osp.tile([128, DK], f32, tag="o_sb")
                    nc.vector.tensor_scalar_mul(o_sb[:], op[:, 0:DK], rcp[:])
                    r0 = a * 512 + ss * 128
                    nc.sync.dma_start(out_d.ap()[r0:r0 + 128, :], o_sb[:])

    nc.compile()
    return nc


def _get_nc(s=S, mm_dtype="float32r"):
    key = (s, mm_dtype)
    if key not in _CACHE:
        _CACHE[key] = build_nc(s, mm_dtype)
    return _CACHE[key]


def kernel(query, key, value, mask, W_q, b_q, W_k, b_k, W_v, b_v):
    from concourse.bass_utils import run_bass_kernel_spmd

    query = np.asarray(query)
    key = np.asarray(key)
    value = np.asarray(value)
    mask = np.asarray(mask)
    B = query.shape[0]
    nc = _get_nc()

    in_maps = []
    for b in range(B):
        in_maps.append({
            "xq": np.ascontiguousarray(query[b]),
            "xk": np.ascontiguousarray(key[b]),
            "xv": np.ascontiguousarray(value[b]),
            "mask": np.ascontiguousarray(mask[b].reshape(1, -1).view(np.uint8)),
            "wq": np.ascontiguousarray(W_q),
            "wk": np.ascontiguousarray(W_k),
            "wv": np.ascontiguousarray(W_v),
            "bq": np.ascontiguousarray(np.asarray(b_q).reshape(1, -1)),
            "bk": np.ascontiguousarray(np.asarray(b_k).reshape(1, -1)),
            "bv": np.ascontiguousarray(np.asarray(b_v).reshape(1, -1)),
        })

    res = run_bass_kernel_spmd(nc, in_maps, core_ids=list(range(B)))
    out = np.stack([res.results[b]["out"] for b in range(B)], axis=0)
    return out.astype(np.float32)
